# revision 30
# baseline (speedup 1.0000x reference)
"""Bahdanau attention on 8 Trainium2 NeuronCores.

  q = ht @ Wa_w.T + Wa_b            [B,1,H]
  k = hs @ Ua_w.T + Ua_b            [B,S,H]
  e = tanh(q + k)                   [B,S,H]
  scores = e @ Va_w.T (+ Va_b)      [B,S,1] -> [B,1,S]
  attn = softmax(scores)            [B,1,S]   (Va_b drops: softmax shift-inv)
  ctx = attn @ hs                   [B,1,H]

Sharding: data-parallel over batch, 32/8 = 4 batches per core, weights
replicated.  Per core everything is computed in a "transposed" layout
(feature dim on partitions) so that
  - the big matmul k^T = Ua_w @ hs^T streams hs^T as the moving operand,
  - tanh(q + k) fuses the q/bias add into the ACT bias port,
  - scores = Va . e^T is a PSUM-accumulated M=1 matmul,
and the final ctx bmm streams hs in its native layout.  Matmuls run in
fp32r (full-rate fp32 PE mode).
"""

import functools

import numpy as np

import concourse.bass as bass
import concourse.tile as tile
from concourse import bacc, mybir
from concourse.bass_utils import run_bass_kernel_spmd

P = 128          # partitions
NCORES = 8
B = 32           # full batch
NB = B // NCORES  # batches per core = 4
S = 2048
H = 1024
HC = H // P      # 8 contraction chunks
OC = H // P      # 8 output-feature chunks
STW = 512        # s tile width (pass 1, matmul free dim)
NST = S // STW   # 4 s-tiles
NSC = S // P     # 16 s-chunks of 128

F32 = mybir.dt.float32
F32R = mybir.dt.float32r


def _build_nc(repeat: int = 1) -> bass.Bass:
    nc = bacc.Bacc("TRN2", target_bir_lowering=False, debug=False)

    hsT_d = nc.dram_tensor("hsT_in", [NB, H, S], F32R, kind="ExternalInput")
    hs_d = nc.dram_tensor("hs_in", [NB, S, H], F32R, kind="ExternalInput")
    htT_d = nc.dram_tensor("htT_in", [P, HC * NB], F32R, kind="ExternalInput")
    UaT_d = nc.dram_tensor("UaT_in", [H, H], F32R, kind="ExternalInput")
    WaT_d = nc.dram_tensor("WaT_in", [H, H], F32R, kind="ExternalInput")
    VaT_d = nc.dram_tensor("VaT_in", [P, OC], F32R, kind="ExternalInput")
    bb_d = nc.dram_tensor("bb_in", [P, OC], F32, kind="ExternalInput")
    id_d = nc.dram_tensor("id_in", [P, P], F32, kind="ExternalInput")

    ctx_d = nc.dram_tensor("ctx_out", [NB, H], F32, kind="ExternalOutput")
    attn_d = nc.dram_tensor("attn_out", [NB, S], F32, kind="ExternalOutput")

    with tile.TileContext(nc) as tc:
        with (
            tc.tile_pool(name="const", bufs=1) as cpool,
            tc.tile_pool(name="hsT", bufs=3) as hsT_pool,
            tc.tile_pool(name="eT", bufs=3) as eT_pool,
            tc.tile_pool(name="hsn", bufs=5) as hs_pool,
            tc.tile_pool(name="sm", bufs=2) as sm_pool,
            tc.tile_pool(name="rows", bufs=1) as row_pool,
            tc.tile_pool(name="accp", bufs=1) as acc_pool,
            tc.tile_pool(name="psk", bufs=2, space="PSUM") as psum_k,
            tc.tile_pool(name="pss", bufs=2, space="PSUM") as psum_s,
            tc.tile_pool(name="pst", bufs=1, space="PSUM") as psum_t,
            tc.tile_pool(name="psc", bufs=1, space="PSUM") as psum_c,
        ):
            # ---- startup: order DMAs by first-need time --------------------
            UaT_src = UaT_d[:].rearrange("(c p) o -> p c o", p=P)
            WaT_src = WaT_d[:].rearrange("(c p) o -> p c o", p=P)
            bb_sb = cpool.tile([P, OC], F32)
            nc.sync.dma_start(bb_sb[:], bb_d[:])
            VaT_sb = cpool.tile([P, OC], F32R)
            nc.sync.dma_start(VaT_sb[:], VaT_d[:])
            htT_sb = cpool.tile([P, HC * NB], F32R)
            nc.sync.dma_start(htT_sb[:], htT_d[:])
            UaT_sb = cpool.tile([P, HC, H], F32R)
            WaT_lo = hs_pool.tile([P, 4, H], F32R, tag="hsn")
            WaT_hi = hs_pool.tile([P, 4, H], F32R, tag="hsn")
            nc.sync.dma_start(UaT_sb[:, :, 0:P], UaT_src[:, :, 0:P])

            def WaT_part(hc):
                return (WaT_lo if hc < 4 else WaT_hi)[:, hc % 4, :]

            qb_sb = cpool.tile([P, OC, NB], F32)
            id_sb = cpool.tile([P, P], F32)
            ones_sb = cpool.tile([P, 1], F32)
            nc.vector.memset(ones_sb[:], 1.0)
            ones_row = cpool.tile([1, P], F32)
            nc.vector.memset(ones_row[:], 1.0)
            state = {"q_done": False, "id_done": False}

            def emit_q_chunk(oc):
                # per-oc q matmul group, fed by the per-oc WaT slice
                q_ps = psum_k.tile([P, NB], F32, tag="k")
                for hc in range(HC):
                    nc.tensor.matmul(
                        q_ps[:],
                        WaT_part(hc)[:, oc * P:(oc + 1) * P],
                        htT_sb[:, hc * NB:(hc + 1) * NB],
                        start=(hc == 0),
                        stop=(hc == HC - 1),
                    )
                nc.vector.tensor_scalar_add(
                    qb_sb[:, oc, :], q_ps[:], bb_sb[:, oc:oc + 1]
                )

            for _rep in range(repeat):
                for b in range(NB):
                    # ---- pass 1: k^T, tanh, scores ------------------------
                    scores_row = row_pool.tile([1, S], F32, tag="scores")
                    attn_row = row_pool.tile([1, S], F32, tag="attn")
                    nm_vec = sm_pool.tile([1, NST], F32, tag="nmv")
                    sum_vec = sm_pool.tile([1, NST], F32, tag="sumv")
                    at_ps = psum_t.tile([P, NSC], F32, tag="t")
                    atT_raw = sm_pool.tile([P, NSC], F32R, tag="atTr")
                    partials = sm_pool.tile([NST, H], F32R, tag="part")
                    for st in range(NST):
                        first = not state["q_done"]
                        hsT_t = hsT_pool.tile([P, HC, STW], F32R, tag="hsT")
                        src = hsT_d[b].rearrange("(c p) s -> p c s", p=P)[
                            :, :, st * STW:(st + 1) * STW
                        ]
                        if first:
                            for hc in range(HC):
                                nc.sync.dma_start(hsT_t[:, hc, :], src[:, hc, :])
                        else:
                            nc.sync.dma_start(hsT_t[:], src)
                        sc_ps = psum_s.tile([1, STW], F32, tag="s")
                        eT_pend = []
                        for oc in range(OC):
                            if first:
                                # stream the startup weights in need-order
                                nc.sync.dma_start(
                                    WaT_lo[:, :, oc * P:(oc + 1) * P].rearrange(
                                        "p c o -> p c o"
                                    ),
                                    WaT_src[:, 0:4, oc * P:(oc + 1) * P],
                                )
                                nc.sync.dma_start(
                                    WaT_hi[:, :, oc * P:(oc + 1) * P].rearrange(
                                        "p c o -> p c o"
                                    ),
                                    WaT_src[:, 4:8, oc * P:(oc + 1) * P],
                                )
                                if oc + 1 < OC:
                                    nc.sync.dma_start(
                                        UaT_sb[:, :, (oc + 1) * P:(oc + 2) * P],
                                        UaT_src[:, :, (oc + 1) * P:(oc + 2) * P],
                                    )
                            kT_ps = psum_k.tile([P, STW], F32, tag="k")
                            for hc in range(HC):
                                nc.tensor.matmul(
                                    kT_ps[:],
                                    UaT_sb[:, hc, oc * P:(oc + 1) * P],
                                    hsT_t[:, hc, :],
                                    start=(hc == 0),
                                    stop=(hc == HC - 1),
                                )
                            if first:
                                emit_q_chunk(oc)
                                if oc == OC - 1:
                                    state["q_done"] = True
                            eT_t = eT_pool.tile([P, STW], F32R, tag="eT")
                            nc.scalar.activation(
                                eT_t[:],
                                kT_ps[:],
                                mybir.ActivationFunctionType.Tanh,
                                bias=qb_sb[:, oc, b:b + 1],
                            )
                            eT_pend.append(eT_t)
                            if oc >= 1:
                                pc = oc - 1
                                nc.tensor.matmul(
                                    sc_ps[:],
                                    VaT_sb[:, pc:pc + 1],
                                    eT_pend[pc][:],
                                    start=(pc == 0),
                                    stop=False,
                                )
                        nc.tensor.matmul(
                            sc_ps[:],
                            VaT_sb[:, OC - 1:OC],
                            eT_pend[OC - 1][:],
                            start=False,
                            stop=True,
                        )
                        nc.vector.tensor_copy(
                            scores_row[0:1, st * STW:(st + 1) * STW], sc_ps[:]
                        )
                        # incremental softmax: per-tile -max, exp, partial sum
                        nc.vector.reduce_max(
                            nm_vec[0:1, st:st + 1],
                            scores_row[0:1, st * STW:(st + 1) * STW],
                            axis=mybir.AxisListType.X,
                            negate=True,
                        )
                        nc.scalar.activation(
                            attn_row[0:1, st * STW:(st + 1) * STW],
                            scores_row[0:1, st * STW:(st + 1) * STW],
                            mybir.ActivationFunctionType.Exp,
                            bias=nm_vec[0:1, st:st + 1],
                            accum_out=sum_vec[0:1, st:st + 1],
                        )
                        if not state["id_done"]:
                            nc.sync.dma_start(id_sb[:], id_d[:])
                            state["id_done"] = True
                        for c4 in range(4):
                            sc4 = st * 4 + c4
                            nc.tensor.transpose(
                                at_ps[:, sc4:sc4 + 1],
                                attn_row[0:1, sc4 * P:(sc4 + 1) * P],
                                id_sb[0:1, 0:1],
                            )
                        nc.vector.tensor_copy(
                            atT_raw[:, st * 4:(st + 1) * 4],
                            at_ps[:, st * 4:(st + 1) * 4],
                        )
                        if b == NB - 1:
                            # last batch: unscaled ctx partial for this s-tile
                            # runs now, under pass-1; the tail only combines.
                            hs_t3 = hs_pool.tile([P, 4, H], F32R, tag="hsn")
                            nc.sync.dma_start(
                                hs_t3[:],
                                hs_d[b].rearrange("(q p) o -> p q o", p=P)[
                                    :, st * 4:(st + 1) * 4, :
                                ],
                            )
                            part_ps = psum_c.tile([1, H], F32, tag="c")
                            for c4 in range(4):
                                sc4 = st * 4 + c4
                                for hh in range(2):
                                    nc.tensor.matmul(
                                        part_ps[:, hh * 512:(hh + 1) * 512],
                                        atT_raw[:, sc4:sc4 + 1],
                                        hs_t3[:, c4, hh * 512:(hh + 1) * 512],
                                        start=(c4 == 0),
                                        stop=(c4 == 3),
                                    )
                            nc.vector.tensor_copy(
                                partials[st:st + 1, :], part_ps[:]
                            )

                    # ---- softmax tail: combine the per-tile partials ------
                    neg_M = sm_pool.tile([1, 1], F32, tag="negM")
                    nc.vector.tensor_reduce(
                        neg_M[:], nm_vec[:], axis=mybir.AxisListType.X,
                        op=mybir.AluOpType.min,
                    )
                    f_vec = sm_pool.tile([1, NST], F32, tag="fv")
                    # f_st = exp(neg_M - nm_st)
                    diff = sm_pool.tile([1, NST], F32, tag="dv")
                    nc.vector.tensor_scalar_sub(diff[:], nm_vec[:], neg_M[:])
                    nc.scalar.activation(
                        f_vec[:], diff[:], mybir.ActivationFunctionType.Exp, scale=-1.0
                    )
                    sw = sm_pool.tile([1, NST], F32, tag="swv")
                    nc.vector.tensor_mul(sw[:], sum_vec[:], f_vec[:])
                    deno = sm_pool.tile([1, 1], F32, tag="deno")
                    nc.vector.tensor_reduce(
                        deno[:], sw[:], axis=mybir.AxisListType.X,
                        op=mybir.AluOpType.add,
                    )
                    rec = sm_pool.tile([1, 1], F32, tag="rec")
                    nc.vector.reciprocal(rec[:], deno[:])
                    g_vec = sm_pool.tile([1, NST], F32, tag="gv")
                    nc.vector.tensor_scalar_mul(g_vec[:], f_vec[:], rec[:])

                    last = b == NB - 1
                    ctx_half = sm_pool.tile([1, H], F32, tag="ctxh")
                    if last:
                        # g as a K=4 stationary column; combine the partials
                        gc_ps = psum_t.tile([NST, 1], F32, tag="g")
                        nc.tensor.transpose(gc_ps[:], g_vec[:], id_sb[0:1, 0:1])
                        gcol_sb = sm_pool.tile([NST, 1], F32R, tag="gcol")
                        nc.vector.tensor_copy(gcol_sb[:], gc_ps[:])
                        comb_ps = psum_c.tile([1, H], F32, tag="c")
                        for hh in range(2):
                            nc.tensor.matmul(
                                comb_ps[:, hh * 512:(hh + 1) * 512],
                                gcol_sb[:],
                                partials[:, hh * 512:(hh + 1) * 512],
                                start=True,
                                stop=True,
                            )
                            nc.vector.tensor_copy(
                                ctx_half[0:1, hh * 512:(hh + 1) * 512],
                                comb_ps[:, hh * 512:(hh + 1) * 512],
                            )
                            nc.sync.dma_start(
                                ctx_d[b:b + 1, hh * 512:(hh + 1) * 512],
                                ctx_half[0:1, hh * 512:(hh + 1) * 512],
                            )
                        atT_sb = None
                    else:
                        # broadcast g across partitions via a K=1 ones matmul,
                        # then scale the small transposed columns
                        gbc_ps = psum_t.tile([P, NST], F32, tag="g")
                        nc.tensor.matmul(
                            gbc_ps[:], ones_row[:], g_vec[:], start=True, stop=True
                        )
                        gbc_sb = sm_pool.tile([P, NST], F32, tag="gbc")
                        nc.vector.tensor_copy(gbc_sb[:], gbc_ps[:])
                        atT_sb = sm_pool.tile([P, NSC], F32, tag="atT")
                        for st in range(NST):
                            nc.vector.tensor_scalar_mul(
                                atT_sb[:, st * 4:(st + 1) * 4],
                                atT_raw[:, st * 4:(st + 1) * 4],
                                gbc_sb[:, st:st + 1],
                            )

                    # attn output row: scale off the critical path
                    for st in range(NST):
                        nc.vector.tensor_scalar_mul(
                            attn_row[0:1, st * STW:(st + 1) * STW],
                            attn_row[0:1, st * STW:(st + 1) * STW],
                            g_vec[0:1, st:st + 1],
                        )
                    nc.sync.dma_start(attn_d[b:b + 1, :], attn_row[:])

                    # ---- ctx = attn @ hs (batches 0..NB-2 on the DVE) -----
                    if not last:
                        ctx_ps = psum_c.tile([1, H], F32, tag="c")
                        acc = acc_pool.tile([P, H], F32, tag="acc")
                        for sq in range(NSC // 4):
                            hs_t = hs_pool.tile([P, 4, H], F32R, tag="hsn")
                            nc.sync.dma_start(
                                hs_t[:],
                                hs_d[b].rearrange("(q p) o -> p q o", p=P)[
                                    :, sq * 4:(sq + 1) * 4, :
                                ],
                            )
                            for c in range(4):
                                sc = sq * 4 + c
                                if sc == 0:
                                    nc.vector.tensor_scalar_mul(
                                        acc[:], hs_t[:, c, :], atT_sb[:, sc:sc + 1]
                                    )
                                else:
                                    nc.vector.scalar_tensor_tensor(
                                        acc[:],
                                        hs_t[:, c, :],
                                        atT_sb[:, sc:sc + 1],
                                        acc[:],
                                        op0=mybir.AluOpType.mult,
                                        op1=mybir.AluOpType.add,
                                    )
                        for hh in range(2):
                            nc.tensor.matmul(
                                ctx_ps[:, hh * 512:(hh + 1) * 512],
                                ones_sb[:],
                                acc[:, hh * 512:(hh + 1) * 512],
                                start=True,
                                stop=True,
                            )
                    if not last:
                        ctx_t = sm_pool.tile([1, H], F32, tag="ctx")
                        nc.vector.tensor_copy(ctx_t[:], ctx_ps[:])
                        nc.sync.dma_start(ctx_d[b:b + 1, :], ctx_t[:])

    return nc


@functools.lru_cache(maxsize=4)
def _get_nc(repeat: int = 1) -> bass.Bass:
    nc = _build_nc(repeat)
    nc.compile()
    return nc


def _pack_core_inputs(ht, hs, htT_full, UaT, WaT, VaT, bb, ident, core):
    lo = core * NB
    hs_l = hs[lo:lo + NB]
    return {
        "hsT_in": np.ascontiguousarray(hs_l.transpose(0, 2, 1)),
        "hs_in": np.ascontiguousarray(hs_l),
        "htT_in": htT_full[:, :, lo:lo + NB].reshape(P, HC * NB).copy(),
        "UaT_in": UaT,
        "WaT_in": WaT,
        "VaT_in": VaT,
        "bb_in": bb,
        "id_in": ident,
    }


def kernel(ht, hs, Wa_w, Wa_b, Ua_w, Ua_b, Va_w, Va_b):
    ht = np.asarray(ht, np.float32)
    hs = np.asarray(hs, np.float32)

    UaT = np.ascontiguousarray(np.asarray(Ua_w, np.float32).T)
    WaT = np.ascontiguousarray(np.asarray(Wa_w, np.float32).T)
    VaT = np.ascontiguousarray(np.asarray(Va_w, np.float32).reshape(HC, P).T)
    bb = np.ascontiguousarray(
        (np.asarray(Wa_b, np.float32) + np.asarray(Ua_b, np.float32)).reshape(HC, P).T
    )
    ident = np.eye(P, dtype=np.float32)
    # htT_full[p, c, b] = ht[b, 0, c*P + p]
    htT_full = np.ascontiguousarray(ht[:, 0, :].T.reshape(HC, P, B).transpose(1, 0, 2))

    nc = _get_nc()
    in_maps = [
        _pack_core_inputs(ht, hs, htT_full, UaT, WaT, VaT, bb, ident, core)
        for core in range(NCORES)
    ]
    res = run_bass_kernel_spmd(nc, in_maps, list(range(NCORES)))

    ctx = np.concatenate([res.results[i]["ctx_out"] for i in range(NCORES)], axis=0)
    attn = np.concatenate([res.results[i]["attn_out"] for i in range(NCORES)], axis=0)
    return ctx.reshape(B, 1, H), attn.reshape(B, 1, S)


# revision 36
# speedup vs baseline: 5.7604x; 5.7604x over previous
"""Bahdanau attention on 8 Trainium2 NeuronCores.

  q = ht @ Wa_w.T + Wa_b            [B,1,H]
  k = hs @ Ua_w.T + Ua_b            [B,S,H]
  e = tanh(q + k)                   [B,S,H]
  scores = e @ Va_w.T (+ Va_b)      [B,S,1] -> [B,1,S]
  attn = softmax(scores)            [B,1,S]   (Va_b drops: softmax shift-inv)
  ctx = attn @ hs                   [B,1,H]

Sharding: data-parallel over batch, 32/8 = 4 batches per core, weights
replicated.  Per core everything is computed in a "transposed" layout
(feature dim on partitions) so that
  - the big matmul k^T = Ua_w @ hs^T streams hs^T as the moving operand,
  - tanh(q + k) fuses the q/bias add into the ACT bias port,
  - scores = Va . e^T is a PSUM-accumulated M=1 matmul,
and the final ctx bmm streams hs in its native layout.  Matmuls run in
fp32r (full-rate fp32 PE mode).
"""

import functools

import numpy as np

import concourse.bass as bass
import concourse.tile as tile
from concourse import bacc, mybir
from concourse.bass_utils import run_bass_kernel_spmd

P = 128          # partitions
NCORES = 8
B = 32           # full batch
NB = B // NCORES  # batches per core = 4
S = 2048
H = 1024
HC = H // P      # 8 contraction chunks
OC = H // P      # 8 output-feature chunks
STW = 512        # s tile width (pass 1, matmul free dim)
NST = S // STW   # 4 s-tiles
NSC = S // P     # 16 s-chunks of 128

F32 = mybir.dt.float32
F32R = mybir.dt.float32r


def _build_nc(repeat: int = 1) -> bass.Bass:
    nc = bacc.Bacc("TRN2", target_bir_lowering=False, debug=False)

    hsT_d = nc.dram_tensor("hsT_in", [NB, H, S], F32R, kind="ExternalInput")
    hs_d = nc.dram_tensor("hs_in", [NB, S, H], F32R, kind="ExternalInput")
    htT_d = nc.dram_tensor("htT_in", [P, HC * NB], F32R, kind="ExternalInput")
    UaT_d = nc.dram_tensor("UaT_in", [H, H], F32R, kind="ExternalInput")
    WaT_d = nc.dram_tensor("WaT_in", [H, H], F32R, kind="ExternalInput")
    VaT_d = nc.dram_tensor("VaT_in", [P, OC], F32R, kind="ExternalInput")
    bb_d = nc.dram_tensor("bb_in", [P, OC], F32, kind="ExternalInput")
    id_d = nc.dram_tensor("id_in", [P, P], F32, kind="ExternalInput")

    ctx_d = nc.dram_tensor("ctx_out", [NB, H], F32, kind="ExternalOutput")
    attn_d = nc.dram_tensor("attn_out", [NB, S], F32, kind="ExternalOutput")

    with tile.TileContext(nc) as tc:
        with (
            tc.tile_pool(name="const", bufs=1) as cpool,
            tc.tile_pool(name="hsT", bufs=3) as hsT_pool,
            tc.tile_pool(name="eT", bufs=3) as eT_pool,
            tc.tile_pool(name="hsn", bufs=5) as hs_pool,
            tc.tile_pool(name="sm", bufs=2) as sm_pool,
            tc.tile_pool(name="rows", bufs=1) as row_pool,
            tc.tile_pool(name="accp", bufs=1) as acc_pool,
            tc.tile_pool(name="psk", bufs=2, space="PSUM") as psum_k,
            tc.tile_pool(name="pss", bufs=2, space="PSUM") as psum_s,
            tc.tile_pool(name="pst", bufs=1, space="PSUM") as psum_t,
            tc.tile_pool(name="psc", bufs=1, space="PSUM") as psum_c,
        ):
            # ---- startup: order DMAs by first-need time --------------------
            UaT_src = UaT_d[:].rearrange("(c p) o -> p c o", p=P)
            WaT_src = WaT_d[:].rearrange("(c p) o -> p c o", p=P)
            bb_sb = cpool.tile([P, OC], F32)
            nc.sync.dma_start(bb_sb[:], bb_d[:])
            VaT_sb = cpool.tile([P, OC], F32R)
            nc.sync.dma_start(VaT_sb[:], VaT_d[:])
            htT_sb = cpool.tile([P, HC * NB], F32R)
            nc.sync.dma_start(htT_sb[:], htT_d[:])
            UaT_sb = cpool.tile([P, HC, H], F32R)
            WaT_lo = hs_pool.tile([P, 4, H], F32R, tag="hsn")
            WaT_hi = hs_pool.tile([P, 4, H], F32R, tag="hsn")
            nc.sync.dma_start(UaT_sb[:, :, 0:P], UaT_src[:, :, 0:P])

            def WaT_part(hc):
                return (WaT_lo if hc < 4 else WaT_hi)[:, hc % 4, :]

            qb_sb = cpool.tile([P, OC, NB], F32)
            id_sb = cpool.tile([P, P], F32)
            ones_sb = cpool.tile([P, 1], F32)
            nc.vector.memset(ones_sb[:], 1.0)
            ones_row = cpool.tile([1, P], F32)
            nc.vector.memset(ones_row[:], 1.0)
            state = {"q_done": False, "id_done": False}

            def emit_q_chunk(oc):
                # per-oc q matmul group, fed by the per-oc WaT slice
                q_ps = psum_k.tile([P, NB], F32, tag="k")
                for hc in range(HC):
                    nc.tensor.matmul(
                        q_ps[:],
                        WaT_part(hc)[:, oc * P:(oc + 1) * P],
                        htT_sb[:, hc * NB:(hc + 1) * NB],
                        start=(hc == 0),
                        stop=(hc == HC - 1),
                    )
                nc.vector.tensor_scalar_add(
                    qb_sb[:, oc, :], q_ps[:], bb_sb[:, oc:oc + 1]
                )

            for _rep in range(repeat):
                for b in range(NB):
                    # ---- pass 1: k^T, tanh, scores ------------------------
                    scores_row = row_pool.tile([1, S], F32, tag="scores")
                    attn_row = row_pool.tile([1, S], F32, tag="attn")
                    nm_vec = sm_pool.tile([1, NST], F32, tag="nmv")
                    sum_vec = sm_pool.tile([1, NST], F32, tag="sumv")
                    at_ps = psum_t.tile([P, NSC], F32, tag="t")
                    atT_raw = sm_pool.tile([P, NSC], F32, tag="atTr")
                    for st in range(NST):
                        first = not state["q_done"]
                        hsT_t = hsT_pool.tile([P, HC, STW], F32R, tag="hsT")
                        src = hsT_d[b].rearrange("(c p) s -> p c s", p=P)[
                            :, :, st * STW:(st + 1) * STW
                        ]
                        if first:
                            for hc in range(HC):
                                nc.sync.dma_start(hsT_t[:, hc, :], src[:, hc, :])
                        else:
                            nc.sync.dma_start(hsT_t[:], src)
                        sc_ps = psum_s.tile([1, STW], F32, tag="s")
                        eT_pend = []
                        for oc in range(OC):
                            if first:
                                # stream the startup weights in need-order
                                nc.sync.dma_start(
                                    WaT_lo[:, :, oc * P:(oc + 1) * P].rearrange(
                                        "p c o -> p c o"
                                    ),
                                    WaT_src[:, 0:4, oc * P:(oc + 1) * P],
                                )
                                nc.sync.dma_start(
                                    WaT_hi[:, :, oc * P:(oc + 1) * P].rearrange(
                                        "p c o -> p c o"
                                    ),
                                    WaT_src[:, 4:8, oc * P:(oc + 1) * P],
                                )
                                if oc + 1 < OC:
                                    nc.sync.dma_start(
                                        UaT_sb[:, :, (oc + 1) * P:(oc + 2) * P],
                                        UaT_src[:, :, (oc + 1) * P:(oc + 2) * P],
                                    )
                            kT_ps = psum_k.tile([P, STW], F32, tag="k")
                            for hc in range(HC):
                                nc.tensor.matmul(
                                    kT_ps[:],
                                    UaT_sb[:, hc, oc * P:(oc + 1) * P],
                                    hsT_t[:, hc, :],
                                    start=(hc == 0),
                                    stop=(hc == HC - 1),
                                )
                            if first:
                                emit_q_chunk(oc)
                                if oc == OC - 1:
                                    state["q_done"] = True
                            eT_t = eT_pool.tile([P, STW], F32R, tag="eT")
                            nc.scalar.activation(
                                eT_t[:],
                                kT_ps[:],
                                mybir.ActivationFunctionType.Tanh,
                                bias=qb_sb[:, oc, b:b + 1],
                            )
                            eT_pend.append(eT_t)
                            if oc >= 1:
                                pc = oc - 1
                                nc.tensor.matmul(
                                    sc_ps[:],
                                    VaT_sb[:, pc:pc + 1],
                                    eT_pend[pc][:],
                                    start=(pc == 0),
                                    stop=False,
                                )
                        nc.tensor.matmul(
                            sc_ps[:],
                            VaT_sb[:, OC - 1:OC],
                            eT_pend[OC - 1][:],
                            start=False,
                            stop=True,
                        )
                        nc.vector.tensor_copy(
                            scores_row[0:1, st * STW:(st + 1) * STW], sc_ps[:]
                        )
                        # incremental softmax: per-tile -max, exp, partial sum
                        nc.vector.reduce_max(
                            nm_vec[0:1, st:st + 1],
                            scores_row[0:1, st * STW:(st + 1) * STW],
                            axis=mybir.AxisListType.X,
                            negate=True,
                        )
                        nc.scalar.activation(
                            attn_row[0:1, st * STW:(st + 1) * STW],
                            scores_row[0:1, st * STW:(st + 1) * STW],
                            mybir.ActivationFunctionType.Exp,
                            bias=nm_vec[0:1, st:st + 1],
                            accum_out=sum_vec[0:1, st:st + 1],
                        )
                        if not state["id_done"]:
                            nc.sync.dma_start(id_sb[:], id_d[:])
                            state["id_done"] = True
                        for c4 in range(4):
                            sc4 = st * 4 + c4
                            nc.tensor.transpose(
                                at_ps[:, sc4:sc4 + 1],
                                attn_row[0:1, sc4 * P:(sc4 + 1) * P],
                                id_sb[0:1, 0:1],
                            )
                        nc.vector.tensor_copy(
                            atT_raw[:, st * 4:(st + 1) * 4],
                            at_ps[:, st * 4:(st + 1) * 4],
                        )

                    # ---- softmax tail: combine the per-tile partials ------
                    neg_M = sm_pool.tile([1, 1], F32, tag="negM")
                    nc.vector.tensor_reduce(
                        neg_M[:], nm_vec[:], axis=mybir.AxisListType.X,
                        op=mybir.AluOpType.min,
                    )
                    f_vec = sm_pool.tile([1, NST], F32, tag="fv")
                    # f_st = exp(neg_M - nm_st)
                    diff = sm_pool.tile([1, NST], F32, tag="dv")
                    nc.vector.tensor_scalar_sub(diff[:], nm_vec[:], neg_M[:])
                    nc.scalar.activation(
                        f_vec[:], diff[:], mybir.ActivationFunctionType.Exp, scale=-1.0
                    )
                    sw = sm_pool.tile([1, NST], F32, tag="swv")
                    nc.vector.tensor_mul(sw[:], sum_vec[:], f_vec[:])
                    deno = sm_pool.tile([1, 1], F32, tag="deno")
                    nc.vector.tensor_reduce(
                        deno[:], sw[:], axis=mybir.AxisListType.X,
                        op=mybir.AluOpType.add,
                    )
                    rec = sm_pool.tile([1, 1], F32, tag="rec")
                    nc.vector.reciprocal(rec[:], deno[:])
                    g_vec = sm_pool.tile([1, NST], F32, tag="gv")
                    nc.vector.tensor_scalar_mul(g_vec[:], f_vec[:], rec[:])

                    # broadcast g across partitions via a K=1 ones matmul,
                    # then scale the small transposed columns (critical path)
                    last = b == NB - 1
                    gbc_ps = psum_t.tile([P, NST], F32, tag="g")
                    nc.tensor.matmul(
                        gbc_ps[:], ones_row[:], g_vec[:], start=True, stop=True
                    )
                    gbc_sb = sm_pool.tile([P, NST], F32, tag="gbc")
                    nc.vector.tensor_copy(gbc_sb[:], gbc_ps[:])
                    atT_sb = sm_pool.tile([P, NSC], F32R if last else F32, tag="atT")
                    ctx_half = sm_pool.tile([1, H], F32, tag="ctxh")
                    for st in range(NST):
                        nc.vector.tensor_scalar_mul(
                            atT_sb[:, st * 4:(st + 1) * 4],
                            atT_raw[:, st * 4:(st + 1) * 4],
                            gbc_sb[:, st:st + 1],
                        )

                    # attn output row: scale off the critical path
                    for st in range(NST):
                        nc.vector.tensor_scalar_mul(
                            attn_row[0:1, st * STW:(st + 1) * STW],
                            attn_row[0:1, st * STW:(st + 1) * STW],
                            g_vec[0:1, st:st + 1],
                        )
                    nc.sync.dma_start(attn_d[b:b + 1, :], attn_row[:])

                    # ---- ctx = attn @ hs ----------------------------------
                    # Batches 0..NB-2: DVE fused mul-add chains (overlap under
                    # the next batch's pass-1 PE work) + a tiny ones-matmul
                    # partition reduce.  Last batch: straight PE matmuls (the
                    # shortest serial tail).
                    ctx_ps = psum_c.tile([1, H], F32, tag="c")
                    if last:
                        hs_ts = []
                        for sq in range(NSC // 4):
                            hs_t = hs_pool.tile([P, 4, H], F32R, tag="hsn")
                            nc.sync.dma_start(
                                hs_t[:],
                                hs_d[b].rearrange("(q p) o -> p q o", p=P)[
                                    :, sq * 4:(sq + 1) * 4, :
                                ],
                            )
                            hs_ts.append(hs_t)
                        # half-major: finish h-half 0 first so its copy/DMA
                        # overlaps the second half's matmuls
                        for hh in range(2):
                            for sq in range(NSC // 4):
                                for c in range(4):
                                    sc = sq * 4 + c
                                    nc.tensor.matmul(
                                        ctx_ps[:, hh * 512:(hh + 1) * 512],
                                        atT_sb[:, sc:sc + 1],
                                        hs_ts[sq][:, c, hh * 512:(hh + 1) * 512],
                                        start=(sc == 0),
                                        stop=(sc == NSC - 1),
                                    )
                            nc.vector.tensor_copy(
                                ctx_half[0:1, hh * 512:(hh + 1) * 512],
                                ctx_ps[:, hh * 512:(hh + 1) * 512],
                            )
                            nc.sync.dma_start(
                                ctx_d[b:b + 1, hh * 512:(hh + 1) * 512],
                                ctx_half[0:1, hh * 512:(hh + 1) * 512],
                            )
                    else:
                        acc = acc_pool.tile([P, H], F32, tag="acc")
                        for sq in range(NSC // 4):
                            hs_t = hs_pool.tile([P, 4, H], F32R, tag="hsn")
                            nc.sync.dma_start(
                                hs_t[:],
                                hs_d[b].rearrange("(q p) o -> p q o", p=P)[
                                    :, sq * 4:(sq + 1) * 4, :
                                ],
                            )
                            for c in range(4):
                                sc = sq * 4 + c
                                if sc == 0:
                                    nc.vector.tensor_scalar_mul(
                                        acc[:], hs_t[:, c, :], atT_sb[:, sc:sc + 1]
                                    )
                                else:
                                    nc.vector.scalar_tensor_tensor(
                                        acc[:],
                                        hs_t[:, c, :],
                                        atT_sb[:, sc:sc + 1],
                                        acc[:],
                                        op0=mybir.AluOpType.mult,
                                        op1=mybir.AluOpType.add,
                                    )
                        for hh in range(2):
                            nc.tensor.matmul(
                                ctx_ps[:, hh * 512:(hh + 1) * 512],
                                ones_sb[:],
                                acc[:, hh * 512:(hh + 1) * 512],
                                start=True,
                                stop=True,
                            )
                    if not last:
                        ctx_t = sm_pool.tile([1, H], F32, tag="ctx")
                        nc.vector.tensor_copy(ctx_t[:], ctx_ps[:])
                        nc.sync.dma_start(ctx_d[b:b + 1, :], ctx_t[:])

    return nc


@functools.lru_cache(maxsize=4)
def _get_nc(repeat: int = 1) -> bass.Bass:
    nc = _build_nc(repeat)
    nc.compile()
    return nc


def _pack_core_inputs(ht, hs, htT_full, UaT, WaT, VaT, bb, ident, core):
    lo = core * NB
    hs_l = hs[lo:lo + NB]
    return {
        "hsT_in": np.ascontiguousarray(hs_l.transpose(0, 2, 1)),
        "hs_in": np.ascontiguousarray(hs_l),
        "htT_in": htT_full[:, :, lo:lo + NB].reshape(P, HC * NB).copy(),
        "UaT_in": UaT,
        "WaT_in": WaT,
        "VaT_in": VaT,
        "bb_in": bb,
        "id_in": ident,
    }


def kernel(ht, hs, Wa_w, Wa_b, Ua_w, Ua_b, Va_w, Va_b):
    ht = np.asarray(ht, np.float32)
    hs = np.asarray(hs, np.float32)

    UaT = np.ascontiguousarray(np.asarray(Ua_w, np.float32).T)
    WaT = np.ascontiguousarray(np.asarray(Wa_w, np.float32).T)
    VaT = np.ascontiguousarray(np.asarray(Va_w, np.float32).reshape(HC, P).T)
    bb = np.ascontiguousarray(
        (np.asarray(Wa_b, np.float32) + np.asarray(Ua_b, np.float32)).reshape(HC, P).T
    )
    ident = np.eye(P, dtype=np.float32)
    # htT_full[p, c, b] = ht[b, 0, c*P + p]
    htT_full = np.ascontiguousarray(ht[:, 0, :].T.reshape(HC, P, B).transpose(1, 0, 2))

    nc = _get_nc()
    in_maps = [
        _pack_core_inputs(ht, hs, htT_full, UaT, WaT, VaT, bb, ident, core)
        for core in range(NCORES)
    ]
    res = run_bass_kernel_spmd(nc, in_maps, list(range(NCORES)))

    ctx = np.concatenate([res.results[i]["ctx_out"] for i in range(NCORES)], axis=0)
    attn = np.concatenate([res.results[i]["attn_out"] for i in range(NCORES)], axis=0)
    return ctx.reshape(B, 1, H), attn.reshape(B, 1, S)
